# revision 21
# baseline (speedup 1.0000x reference)
"""Trainium2 Bass kernel for AttentionNet:
out[b,h,i,j] = relu(sum_d w2[d] * Xf[b,h,i,d] * Yf[b,h,j,d] + b2)
where Xf = X @ W1.T + b1, Yf = Y @ W1.T + b1.

Shapes (hardcoded): X,Y [8, 4, 1024, 64] f32; W1 [64,64]; b1,w2 [64]; b2 [].
Sharding: data-parallel over the fused B*H=32 head dim -> 4 heads per core
across 8 NeuronCores; W1/b1/w2/b2 replicated.

Device plan per core (4 heads = 2 head-pairs):
- heads are processed in pairs packed into the two 64-row halves of the
  128-partition dim, so all K=64 matmuls run 2x concurrent on the PE
  via tile_position row groups.
- per pair and tensor (X, Y): DMA [1024, 64] heads side by side,
  PE-transpose 8x [128, 128] -> [128, 1024] PSUM (rows 0-63 = head0.T,
  64-127 = head1.T), DVE copy to SBUF (rounding to f32r), one matmul
  pair with replicated W1.T applies lin1, ACT fuses (x+b1)*w2 / (y+b1)
  on the PSUM->SBUF copy.
- scores: lhsT = A.T[64, 128-chunk], rhs = B.T[64, 512-chunk] in f32r
  (TF32) at 1 cycle/row; relu(x+b2) on the PSUM->SBUF evacuation
  (alternating ACT/DVE); DMA out on the sync HWDGE ring while input
  loads use the scalar ring.
"""

import ml_dtypes
import numpy as np
from contextlib import ExitStack

import concourse.bass as bass
import concourse.tile as tile
from concourse import bacc, mybir
from concourse.bass_utils import run_bass_kernel_spmd

B, H, L, D = 8, 4, 1024, 64
NCORES = 8
HPC = (B * H) // NCORES  # heads per core = 4

F32 = mybir.dt.float32
MM_DT = mybir.dt.bfloat16


def _mm(ap):
    """Matmul-operand view; with bf16 tiles the cast happens in the
    producing op, so this is the identity."""
    return ap


LAST_RESULT = None
_CACHED_NC = None


def _build():
    nc = bacc.Bacc()
    Xd = nc.declare_dram_parameter("X", [HPC, L, D], F32, isOutput=False)
    Yd = nc.declare_dram_parameter("Y", [HPC, L, D], F32, isOutput=False)
    W1T2d = nc.declare_dram_parameter("W1T2", [128, D], F32, isOutput=False)
    Cd = nc.declare_dram_parameter("CONSTS", [128, 4], F32, isOutput=False)
    Idd = nc.declare_dram_parameter("IDENT", [128, 128], MM_DT, isOutput=False)
    Od = nc.declare_dram_parameter("OUT", [HPC, L, L], F32, isOutput=True)

    AF = mybir.ActivationFunctionType

    with tile.TileContext(nc) as tc, ExitStack() as ctx:
        cpool = ctx.enter_context(tc.tile_pool(name="consts", bufs=1))
        xin_pool = ctx.enter_context(tc.tile_pool(name="xin", bufs=2))
        xbf_pool = ctx.enter_context(tc.tile_pool(name="xbf", bufs=2))
        xt_pool = ctx.enter_context(tc.tile_pool(name="xt", bufs=2))
        ab_pool = ctx.enter_context(tc.tile_pool(name="ab", bufs=4))
        out_pool = ctx.enter_context(tc.tile_pool(name="out", bufs=6))
        pt_pool = ctx.enter_context(tc.tile_pool(name="pt", bufs=2, space="PSUM"))
        pf_pool = ctx.enter_context(tc.tile_pool(name="pf", bufs=1, space="PSUM"))
        ps_pool = ctx.enter_context(tc.tile_pool(name="ps", bufs=2, space="PSUM"))

        w1t2_raw = cpool.tile([128, D], F32, tag="w1t2_raw")
        nc.scalar.dma_start(w1t2_raw[:, :], W1T2d[:, :])
        w1t2 = cpool.tile([128, D], MM_DT, tag="w1t2")
        nc.vector.tensor_copy(_mm(w1t2[:, :]), w1t2_raw[:, :])
        consts = cpool.tile([128, 4], F32, tag="consts")
        nc.scalar.dma_start(consts[:, :], Cd[:, :])
        ident = cpool.tile([128, 128], MM_DT, tag="ident")
        nc.scalar.dma_start(ident[:, :], Idd[:, :])
        # consts columns: 0 = b1*w2 (stacked 2x), 1 = w2 (2x), 2 = b1 (2x),
        # 3 = b2 broadcast
        biasx = consts[:, 0:1]
        scalex = consts[:, 1:2]
        biasy = consts[:, 2:3]
        b2col = consts[:, 3:4]

        relu_ctr = 0
        for pair in range(HPC // 2):
            h0 = 2 * pair
            ab = {}
            for nm, src, bias_ap, scale_ap in (
                ("a", Xd, biasx, scalex),
                ("b", Yd, biasy, None),
            ):
                # natural-layout load: one DMA, 2 KiB contiguous per
                # partition. xin[p, (s r d)] = src[h0+s, 8p + r, d]
                xin = xin_pool.tile([128, 8 * 2 * D], F32, tag="xin")
                nc.scalar.dma_start(
                    xin[:, :].rearrange("p (s r d) -> p s r d", s=2, r=8),
                    src[h0 : h0 + 2, :, :].rearrange(
                        "s (p r) d -> p s r d", r=8
                    ),
                )
                # cast to bf16, permuting free dims to (r, s, d) so each
                # r0-block is a contiguous [128, (s d)] transpose input
                xbf = xbf_pool.tile([128, 8 * 2 * D], MM_DT, tag="xbf")
                nc.vector.tensor_copy(
                    xbf[:, :].rearrange("p (r s d) -> p r s d", s=2, r=8),
                    xin[:, :].rearrange("p (s r d) -> p r s d", s=2, r=8),
                )
                # PE transpose block r0 -> [128, 128] PSUM: rows 0-63 =
                # head0 d's, 64-127 = head1 d's; columns are i = 8p + r0
                pt = pt_pool.tile([128, L], MM_DT, tag="pt")
                for r0 in range(8):
                    nc.tensor.transpose(
                        pt[:, bass.ts(r0, 128)], xbf[:, bass.ts(r0, 128)], ident[:, :]
                    )
                xt = xt_pool.tile([128, L], MM_DT, tag="xt")
                nc.vector.tensor_copy(_mm(xt[:, :]), pt[:, :])
                # lin1 for both heads concurrently on row groups 0-1 / 2-3
                pf = pf_pool.tile([128, L], F32, tag="pf")
                for s in range(2):
                    rows = slice(64 * s, 64 * s + 64)
                    for n in range(2):
                        nc.tensor.matmul(
                            pf[rows, bass.ts(n, 512)],
                            lhsT=_mm(w1t2[rows, :]),
                            rhs=_mm(xt[rows, bass.ts(n, 512)]),
                            start=True,
                            stop=True,
                            tile_position=(64 * s, 64 * s),
                        )
                # fused (x + b1) * w2  (resp. y + b1) on PSUM->SBUF copy
                dst = ab_pool.tile([128, L], MM_DT, tag="ab")
                nc.scalar.activation(
                    _mm(dst[:, :]),
                    pf[:, :],
                    AF.Identity,
                    bias=bias_ap,
                    scale=scale_ap if scale_ap is not None else 1.0,
                )
                ab[nm] = dst
            # scores: out[i, j] = sum_d A.T[d, i] * B.T[d, j]; the two
            # heads of the pair run on disjoint PE row groups. lhsT block
            # m covers rows i = 8p + m; the rhs AP re-orders stored
            # columns (r', p') into ascending j = 8p' + r' so the PSUM
            # tile comes out j-contiguous.
            for m in range(8):
                for s in range(2):
                    rows = slice(64 * s, 64 * s + 64)
                    bb = ab["b"][rows, :].rearrange("k (r p) -> k p r", r=8)
                    ps = ps_pool.tile([128, L], F32, tag="ps")
                    for n in range(2):
                        nc.tensor.matmul(
                            ps[:, bass.ts(n, 512)],
                            lhsT=_mm(ab["a"][rows, bass.ts(m, 128)]),
                            rhs=_mm(bb[:, bass.ts(n, 64), :]),
                            start=True,
                            stop=True,
                            tile_position=(64 * s, 0),
                        )
                    o = out_pool.tile([128, L], F32, tag="o")
                    if relu_ctr % 2 == 0:
                        nc.scalar.activation(
                            o[:, :], ps[:, :], AF.Relu, bias=b2col, scale=1.0
                        )
                    else:
                        nc.vector.tensor_scalar(
                            o[:, :],
                            ps[:, :],
                            b2col,
                            0.0,
                            mybir.AluOpType.add,
                            mybir.AluOpType.max,
                        )
                    relu_ctr += 1
                    # A.T block m has columns i = 8p + m, so scores rows
                    # scatter back with partition stride 8.
                    nc.sync.dma_start(
                        Od[h0 + s, :, :].rearrange("(p r) j -> p r j", r=8)[
                            :, m, :
                        ],
                        o[:, :],
                    )
    nc.compile()
    return nc


def kernel(X, Y, W1, b1, w2, b2):
    global LAST_RESULT, _CACHED_NC
    X = np.ascontiguousarray(np.asarray(X), dtype=np.float32).reshape(B * H, L, D)
    Y = np.ascontiguousarray(np.asarray(Y), dtype=np.float32).reshape(B * H, L, D)
    W1 = np.asarray(W1, dtype=np.float32)
    b1 = np.asarray(b1, dtype=np.float32)
    w2 = np.asarray(w2, dtype=np.float32)
    b2v = float(np.asarray(b2))

    W1T2 = np.ascontiguousarray(np.vstack([W1.T, W1.T]), dtype=np.float32)
    consts = np.ascontiguousarray(
        np.stack(
            [
                np.tile(b1 * w2, 2),
                np.tile(w2, 2),
                np.tile(b1, 2),
                np.full(128, b2v, np.float32),
            ],
            axis=1,
        ),
        dtype=np.float32,
    )
    ident = np.eye(128, dtype=ml_dtypes.bfloat16)

    if _CACHED_NC is None:
        _CACHED_NC = _build()
    nc = _CACHED_NC

    in_maps = [
        {
            "X": np.ascontiguousarray(X[i * HPC : (i + 1) * HPC]),
            "Y": np.ascontiguousarray(Y[i * HPC : (i + 1) * HPC]),
            "W1T2": W1T2,
            "CONSTS": consts,
            "IDENT": ident,
        }
        for i in range(NCORES)
    ]
    res = run_bass_kernel_spmd(nc, in_maps, list(range(NCORES)))
    LAST_RESULT = res
    out = np.concatenate([res.results[i]["OUT"] for i in range(NCORES)], axis=0)
    return out.reshape(B, H, L, L)


# revision 23
# speedup vs baseline: 1.1231x; 1.1231x over previous
"""Trainium2 Bass kernel for AttentionNet:
out[b,h,i,j] = relu(sum_d w2[d] * Xf[b,h,i,d] * Yf[b,h,j,d] + b2)
where Xf = X @ W1.T + b1, Yf = Y @ W1.T + b1.

Shapes (hardcoded): X,Y [8, 4, 1024, 64] f32; W1 [64,64]; b1,w2 [64]; b2 [].
Sharding: data-parallel over the fused B*H=32 head dim -> 4 heads per core
across 8 NeuronCores; W1/b1/w2/b2 replicated.

Device plan per core (4 heads = 2 head-pairs):
- heads are processed in pairs packed into the two 64-row halves of the
  128-partition dim, so all K=64 matmuls run 2x concurrent on the PE
  via tile_position row groups.
- per pair and tensor (X, Y): DMA [1024, 64] heads side by side,
  PE-transpose 8x [128, 128] -> [128, 1024] PSUM (rows 0-63 = head0.T,
  64-127 = head1.T), DVE copy to SBUF (rounding to f32r), one matmul
  pair with replicated W1.T applies lin1, ACT fuses (x+b1)*w2 / (y+b1)
  on the PSUM->SBUF copy.
- scores: lhsT = A.T[64, 128-chunk], rhs = B.T[64, 512-chunk] in f32r
  (TF32) at 1 cycle/row; relu(x+b2) on the PSUM->SBUF evacuation
  (alternating ACT/DVE); DMA out on the sync HWDGE ring while input
  loads use the scalar ring.
"""

import ml_dtypes
import numpy as np
from contextlib import ExitStack

import concourse.bass as bass
import concourse.tile as tile
from concourse import bacc, mybir
from concourse.bass_utils import run_bass_kernel_spmd

B, H, L, D = 8, 4, 1024, 64
NCORES = 8
HPC = (B * H) // NCORES  # heads per core = 4

F32 = mybir.dt.float32
MM_DT = mybir.dt.bfloat16


def _mm(ap):
    """Matmul-operand view; with bf16 tiles the cast happens in the
    producing op, so this is the identity."""
    return ap


LAST_RESULT = None
_CACHED_NC = None


def _build():
    nc = bacc.Bacc()
    Xd = nc.declare_dram_parameter("X", [HPC, L, D], F32, isOutput=False)
    Yd = nc.declare_dram_parameter("Y", [HPC, L, D], F32, isOutput=False)
    W1T2d = nc.declare_dram_parameter("W1T2", [128, D], F32, isOutput=False)
    Cd = nc.declare_dram_parameter("CONSTS", [128, 4], F32, isOutput=False)
    Idd = nc.declare_dram_parameter("IDENT", [128, 128], MM_DT, isOutput=False)
    Od = nc.declare_dram_parameter("OUT", [HPC, L, L], F32, isOutput=True)

    AF = mybir.ActivationFunctionType

    with tile.TileContext(nc) as tc, ExitStack() as ctx:
        cpool = ctx.enter_context(tc.tile_pool(name="consts", bufs=1))
        xin_pool = ctx.enter_context(tc.tile_pool(name="xin", bufs=2))
        xbf_pool = ctx.enter_context(tc.tile_pool(name="xbf", bufs=2))
        xt_pool = ctx.enter_context(tc.tile_pool(name="xt", bufs=2))
        ab_pool = ctx.enter_context(tc.tile_pool(name="ab", bufs=4))
        out_pool = ctx.enter_context(tc.tile_pool(name="out", bufs=6))
        pt_pool = ctx.enter_context(tc.tile_pool(name="pt", bufs=2, space="PSUM"))
        pf_pool = ctx.enter_context(tc.tile_pool(name="pf", bufs=1, space="PSUM"))
        ps_pool = ctx.enter_context(tc.tile_pool(name="ps", bufs=2, space="PSUM"))

        w1t2_raw = cpool.tile([128, D], F32, tag="w1t2_raw")
        nc.scalar.dma_start(w1t2_raw[:, :], W1T2d[:, :])
        w1t2 = cpool.tile([128, D], MM_DT, tag="w1t2")
        nc.vector.tensor_copy(_mm(w1t2[:, :]), w1t2_raw[:, :])
        consts = cpool.tile([128, 4], F32, tag="consts")
        nc.scalar.dma_start(consts[:, :], Cd[:, :])
        ident = cpool.tile([128, 128], MM_DT, tag="ident")
        nc.scalar.dma_start(ident[:, :], Idd[:, :])
        # consts columns: 0 = b1*w2 (stacked 2x), 1 = w2 (2x), 2 = b1 (2x),
        # 3 = b2 broadcast
        biasx = consts[:, 0:1]
        scalex = consts[:, 1:2]
        biasy = consts[:, 2:3]
        b2col = consts[:, 3:4]

        relu_ctr = 0
        for pair in range(HPC // 2):
            h0 = 2 * pair
            ab = {}
            for nm, src, bias_ap, scale_ap in (
                ("a", Xd, biasx, scalex),
                ("b", Yd, biasy, None),
            ):
                xin = xin_pool.tile([128, 8 * 2 * D], F32, tag="xin")
                xbf = xbf_pool.tile([128, 8 * 2 * D], MM_DT, tag="xbf")
                if nm == "a":
                    # X side: natural-layout load -- one DMA, 2 KiB
                    # contiguous per partition: xin[p, (s r d)] =
                    # src[h0+s, 8p + r, d]. The cast permutes free dims
                    # to (r, s, d) so each r0-block is a contiguous
                    # [128, (s d)] transpose input. Transposed columns
                    # land in i = 8p + r0 order; the out-DMA partition
                    # stride undoes the permutation.
                    nc.scalar.dma_start(
                        xin[:, :].rearrange("p (s r d) -> p s r d", s=2, r=8),
                        src[h0 : h0 + 2, :, :].rearrange(
                            "s (p r) d -> p s r d", r=8
                        ),
                    )
                    nc.vector.tensor_copy(
                        xbf[:, :].rearrange("p (r s d) -> p r s d", s=2, r=8),
                        xin[:, :].rearrange("p (s r d) -> p r s d", s=2, r=8),
                    )
                else:
                    # Y side must end up j-contiguous (the scores rhs and
                    # the output free dim follow its column order), so
                    # load [p, (c s d)] = src[h0+s, c*128 + p, d] directly
                    # (256 B descriptors, Y only) and cast straight.
                    xin4 = xin[:, :].rearrange("p (c s d) -> p c s d", c=8, s=2)
                    for s in range(2):
                        nc.scalar.dma_start(
                            xin4[:, :, s, :],
                            src[h0 + s, :, :].rearrange(
                                "(c p) d -> p c d", p=128
                            ),
                        )
                    nc.vector.tensor_copy(xbf[:, :], xin[:, :])
                # PE transpose block k -> [128, 128] PSUM: rows 0-63 =
                # head0 d's, 64-127 = head1 d's
                pt = pt_pool.tile([128, L], MM_DT, tag="pt")
                for k in range(8):
                    nc.tensor.transpose(
                        pt[:, bass.ts(k, 128)], xbf[:, bass.ts(k, 128)], ident[:, :]
                    )
                xt = xt_pool.tile([128, L], MM_DT, tag="xt")
                nc.vector.tensor_copy(_mm(xt[:, :]), pt[:, :])
                # lin1 for both heads concurrently on row groups 0-1 / 2-3
                pf = pf_pool.tile([128, L], F32, tag="pf")
                for s in range(2):
                    rows = slice(64 * s, 64 * s + 64)
                    for n in range(2):
                        nc.tensor.matmul(
                            pf[rows, bass.ts(n, 512)],
                            lhsT=_mm(w1t2[rows, :]),
                            rhs=_mm(xt[rows, bass.ts(n, 512)]),
                            start=True,
                            stop=True,
                            tile_position=(64 * s, 64 * s),
                        )
                # fused (x + b1) * w2  (resp. y + b1) on PSUM->SBUF copy
                dst = ab_pool.tile([128, L], MM_DT, tag="ab")
                nc.scalar.activation(
                    _mm(dst[:, :]),
                    pf[:, :],
                    AF.Identity,
                    bias=bias_ap,
                    scale=scale_ap if scale_ap is not None else 1.0,
                )
                ab[nm] = dst
            # scores: out[i, j] = sum_d A.T[d, i] * B.T[d, j]; the two
            # heads of the pair run on disjoint PE row groups. lhsT block
            # m covers rows i = 8p + m; rhs columns are j-contiguous.
            for m in range(8):
                for s in range(2):
                    rows = slice(64 * s, 64 * s + 64)
                    ps = ps_pool.tile([128, L], F32, tag="ps")
                    for n in range(2):
                        nc.tensor.matmul(
                            ps[:, bass.ts(n, 512)],
                            lhsT=_mm(ab["a"][rows, bass.ts(m, 128)]),
                            rhs=_mm(ab["b"][rows, bass.ts(n, 512)]),
                            start=True,
                            stop=True,
                            tile_position=(64 * s, 0),
                        )
                    o = out_pool.tile([128, L], F32, tag="o")
                    if relu_ctr % 2 == 0:
                        nc.scalar.activation(
                            o[:, :], ps[:, :], AF.Relu, bias=b2col, scale=1.0
                        )
                    else:
                        nc.vector.tensor_scalar(
                            o[:, :],
                            ps[:, :],
                            b2col,
                            0.0,
                            mybir.AluOpType.add,
                            mybir.AluOpType.max,
                        )
                    relu_ctr += 1
                    # A.T block m has columns i = 8p + m, so scores rows
                    # scatter back with partition stride 8.
                    nc.sync.dma_start(
                        Od[h0 + s, :, :].rearrange("(p r) j -> p r j", r=8)[
                            :, m, :
                        ],
                        o[:, :],
                    )
    nc.compile()
    return nc


def kernel(X, Y, W1, b1, w2, b2):
    global LAST_RESULT, _CACHED_NC
    X = np.ascontiguousarray(np.asarray(X), dtype=np.float32).reshape(B * H, L, D)
    Y = np.ascontiguousarray(np.asarray(Y), dtype=np.float32).reshape(B * H, L, D)
    W1 = np.asarray(W1, dtype=np.float32)
    b1 = np.asarray(b1, dtype=np.float32)
    w2 = np.asarray(w2, dtype=np.float32)
    b2v = float(np.asarray(b2))

    W1T2 = np.ascontiguousarray(np.vstack([W1.T, W1.T]), dtype=np.float32)
    consts = np.ascontiguousarray(
        np.stack(
            [
                np.tile(b1 * w2, 2),
                np.tile(w2, 2),
                np.tile(b1, 2),
                np.full(128, b2v, np.float32),
            ],
            axis=1,
        ),
        dtype=np.float32,
    )
    ident = np.eye(128, dtype=ml_dtypes.bfloat16)

    if _CACHED_NC is None:
        _CACHED_NC = _build()
    nc = _CACHED_NC

    in_maps = [
        {
            "X": np.ascontiguousarray(X[i * HPC : (i + 1) * HPC]),
            "Y": np.ascontiguousarray(Y[i * HPC : (i + 1) * HPC]),
            "W1T2": W1T2,
            "CONSTS": consts,
            "IDENT": ident,
        }
        for i in range(NCORES)
    ]
    res = run_bass_kernel_spmd(nc, in_maps, list(range(NCORES)))
    LAST_RESULT = res
    out = np.concatenate([res.results[i]["OUT"] for i in range(NCORES)], axis=0)
    return out.reshape(B, H, L, L)
